# revision 18
# baseline (speedup 1.0000x reference)
"""Dynamic lightweight convolution TRN2 kernel — banded-matmul design.

out[b,l,d] = (1/K) * sum_k softmax_k(x[b,l+K-1,:] @ W + bias)[k, d%H] * x[b,l+k,d]

B=8, S=2048, D=1024, K=7, H=16, L=S-K+1=2042.
Sharding: data-parallel over batch, one batch element per NeuronCore (8 cores).

Per-core plan — the conv itself runs on the *tensor engine* as banded-matrix
matmuls instead of elementwise DVE/GPSIMD work (which bottlenecked the old
design at ~104us busy per engine):

  1. x is loaded by GPSIMD (SWDGE) casting DMAs: f32 HBM -> bf16 SBUF chunks
     xb[i] [128, 1024] in natural [s, d] layout (halves input DMA bytes and
     removes the cast pass entirely).
  2. Logits path (from xb): PE-transpose xb -> xT per s-block, logits =
     W^T @ xT (PE, fp32 psum), e = exp(logits + bias) (ACT), denominators via
     a [112,112] selector matmul (PE), rinv = 1/. (DVE), en = e * rinv (DVE).
     W/bias columns are host-permuted k -> 6-k, so row 16j+h of en is the
     normalized weight of tap k = 6-j.
  3. Per-j-group shifted SBUF->SBUF DMAs build et[16j+h, s] = en[16j+h, s+j]
     (engine copies can't start at partition 16j, DMA can), then
     PE-transposes give T[s, r] (r = 16j+h), stored in T_all [128, chunk, 112].
  4. Band construction via a DRAM bounce (SBUF scatter DMAs cannot skew more
     than 128 bytes across partitions — hw descriptor drift limit — but DRAM
     strides are free): T_all[:, b] is written to a zero-filled DRAM image at
     skewed offsets IMG_SKEW*p + r and read back with row pitch IMG_PITCH,
     which lands T[p, r] at band position (p, 16p + r).  Non-band cells stay
     zero across blocks since each block overwrites exactly the same cells.
  5. Conv for 128-row output block b: for each h, a banded matmul
       out[l, d'] = sum_s A1_h[s, l] * xb[b][s, 16d'+h]
     with stationary A1_h = a1[:, h : h+2048 : 16] (h-interleaved band view).
     The 6-row contraction tail (s in the next chunk) uses a2: its band cells
     are exactly the *left guard* cells of img1(b+1), so a tiny [6, 112] load
     from img1(b+1) (rest of a2 is memset zero once) + a second matmul
     accumulating into the same psum.  PE cost is out-cols * 1 cyc/row only;
     LdWeights are free.
  6. psum [128, 1024] (h-major) -> SBUF staging with a de-interleaving copy
     (dst AP reorders 64h+d' -> 16d'+h); two blocks share one staging tile
     and one paired store DMA (3-dim DRAM dst AP).
"""

import numpy as np
import ml_dtypes
from contextlib import ExitStack

import concourse.bacc as bacc
import concourse.tile as tile
from concourse import mybir
from concourse import bass_utils
from concourse.ap import AP

K = 7
H = 16
B, S, D = 8, 2048, 1024
L = S - K + 1  # 2042
C = D // 128  # 8 d-chunks
NCH = S // 128  # 16 s-chunks
NB = 16  # output blocks of 128 rows (last has 122 valid)
KH = K * H  # 112

SLOT0 = 96  # img col of (l_rel=0, h=0): band tiles are loaded from this col
ACOLS = 2064  # band-tile cols needed by the stationary views
A2LO = 1952  # a2 col of (l_rel=122, h=0); cols below are zero
IMG_PITCH = 2256  # image read pitch (elements)
IMG_SKEW = IMG_PITCH + 16  # image write pitch: +16 elems (one slot) per row
IMG_ELEMS = IMG_PITCH * 128

F32 = mybir.dt.float32
BF16 = mybir.dt.bfloat16

# byte offsets (per partition) inside the packed constants blob
_OFF_BIAS = 0      # [112, 1] f32
_OFF_IDENTB = 4    # [128, 128] bf16
_OFF_SELSUM = 260  # [112, 112] bf16
_OFF_WT = 484      # [128, 8, 112] bf16
_CONST_BYTES = 2276  # 569 f32 columns


def _host_constants(W, b):
    """Pack bias/identb/selsum/W into one [128, 569] f32 blob."""
    buf = np.zeros((128, _CONST_BYTES), np.uint8)

    def put(off, arr):
        by = np.ascontiguousarray(arr).view(np.uint8).reshape(arr.shape[0], -1)
        buf[: arr.shape[0], off : off + by.shape[1]] = by

    # Permute the k-axis (k -> 6-k) of W and bias so that logits/e/en rows
    # come out in j-order (row 16j+h is the weight for tap k=6-j), matching
    # the band-image run layout r = 16j+h.
    perm = np.array([16 * (K - 1 - j) + h for j in range(K) for h in range(H)])
    put(_OFF_BIAS, np.asarray(b, np.float32)[perm].reshape(KH, 1))
    put(_OFF_IDENTB, np.eye(128).astype(ml_dtypes.bfloat16))
    hh = np.arange(KH) % H
    selsum = ((hh[:, None] == hh[None, :]) * float(K)).astype(ml_dtypes.bfloat16)
    put(_OFF_SELSUM, selsum)
    # W [D, KH] -> permuted -> [128, C, KH] chunks (d = c*128 + p)
    wt = np.asarray(W, np.float32)[:, perm].astype(ml_dtypes.bfloat16)
    wt = wt.reshape(C, 128, KH).transpose(1, 0, 2).reshape(128, C * KH)
    put(_OFF_WT, np.ascontiguousarray(wt))
    return buf.view(np.float32)


def build_program():
    nc = bacc.Bacc(
        "TRN2", target_bir_lowering=False, debug=False, enable_asserts=True
    )

    x_d = nc.dram_tensor("x", [S, D], F32, kind="ExternalInput").ap()
    consts_d = nc.dram_tensor(
        "consts", [128, _CONST_BYTES // 4], F32, kind="ExternalInput"
    ).ap()
    out_d = nc.dram_tensor("out", [L, D], F32, kind="ExternalOutput").ap()
    img1 = [
        nc.dram_tensor(f"img1{i}", [IMG_ELEMS], BF16, kind="Internal").ap()
        for i in range(4)
    ]

    with tile.TileContext(nc) as tc, ExitStack() as ctx:
        singles = ctx.enter_context(tc.tile_pool(name="singles", bufs=1))
        xT_pool = ctx.enter_context(tc.tile_pool(name="xT", bufs=2))
        a1_pool = ctx.enter_context(tc.tile_pool(name="a1", bufs=3))
        outs_pool = ctx.enter_context(tc.tile_pool(name="outs", bufs=2))

        p_tp = ctx.enter_context(tc.tile_pool(name="ptp", bufs=2, space="PSUM"))
        p_log = ctx.enter_context(tc.tile_pool(name="plog", bufs=1, space="PSUM"))
        p_sd = ctx.enter_context(tc.tile_pool(name="psd", bufs=1, space="PSUM"))
        p_out = ctx.enter_context(tc.tile_pool(name="pout", bufs=2, space="PSUM"))

        # ---- constants: one packed DMA, tiles are views into the blob ----
        cblob = singles.tile([128, _CONST_BYTES // 4], F32)
        nc.sync.dma_start(out=cblob, in_=consts_d)
        cbytes = cblob.bitcast(mybir.dt.uint8)

        def cview(off, nbytes, dt, rows=128):
            return cbytes[:rows, off : off + nbytes].bitcast(dt)

        bias_t = cview(_OFF_BIAS, 4, F32, rows=KH)
        identb_t = cview(_OFF_IDENTB, 256, BF16)
        selsum_t = cview(_OFF_SELSUM, 224, BF16, rows=KH)
        wt = cview(_OFF_WT, 1792, BF16).rearrange("p (c n) -> p c n", c=C)

        # GPSIMD ucode warmup
        warm = singles.tile([1, 8], BF16)
        nc.gpsimd.tensor_mul(warm, identb_t[:1, :8], identb_t[:1, :8])

        # ---- persistent tensors ----
        xb = [
            singles.tile([128, D], BF16, name=f"xb{i}") for i in range(NCH)
        ]
        e_full = singles.tile([KH, S], BF16)
        rinv = singles.tile([KH, S], F32)
        en = singles.tile([KH, S], BF16)
        et = singles.tile([KH, S], BF16)  # et[16j+h, s] = en[16j+h, s+j]
        t_all = singles.tile([128, NCH, KH], BF16)  # T[s, r], chunked
        zt = singles.tile([128, IMG_PITCH], BF16)  # zeros for image fill
        a2t = [
            singles.tile([6, ACOLS], BF16, name=f"a2t{i}") for i in range(2)
        ]

        # ---- prologue ----
        nc.vector.memset(zt, 0.0)
        # et tail cols: only read for invalid outputs l >= L; keep finite
        nc.vector.memset(et[:, S - 6 :], 0.0)
        # a2 tiles: cols < A2LO are always zero (out-of-band)
        nc.vector.memset(a2t[0], 0.0)
        nc.vector.memset(a2t[1], 0.0)
        for i in range(4):
            nc.sync.dma_start(
                out=AP(tensor=img1[i].tensor, offset=0,
                       ap=[[IMG_PITCH, 128], [1, IMG_PITCH]]),
                in_=zt[:, :],
            )
        # casting input DMAs (f32 HBM -> bf16 SBUF) via GPSIMD SWDGE
        for i in range(NCH):
            nc.gpsimd.dma_start(out=xb[i], in_=x_d[128 * i : 128 * (i + 1), :])

        # ---- stage helpers ----
        def front(sb):
            """Transpose chunks 4sb..4sb+3, logits, exp, denom, rinv, en."""
            sl = slice(512 * sb, 512 * (sb + 1))
            xTt = xT_pool.tile([128, C, 512], BF16, tag="xT")
            for q in range(4):
                i = 4 * sb + q
                ptp = p_tp.tile([128, D], BF16, tag="ptp")
                for c in range(C):
                    nc.tensor.transpose(
                        ptp[:, 128 * c : 128 * (c + 1)],
                        xb[i][:, 128 * c : 128 * (c + 1)],
                        identb_t,
                    )
                eng = nc.vector if q % 2 == 0 else nc.scalar
                cp = (eng.tensor_copy if q % 2 == 0 else eng.copy)
                cp(
                    xTt[:, :, 128 * q : 128 * (q + 1)],
                    ptp.rearrange("p (c s) -> p c s", c=C),
                )
            plog = p_log.tile([KH, 512], F32, tag="plog")
            for c in range(C):
                nc.tensor.matmul(
                    plog, wt[:, c, :], xTt[:, c, :],
                    start=(c == 0), stop=(c == C - 1),
                )
            nc.scalar.activation(
                e_full[:, sl], plog,
                mybir.ActivationFunctionType.Exp, bias=bias_t, scale=1.0,
            )
            psd = p_sd.tile([KH, 512], F32, tag="psd")
            nc.tensor.matmul(psd, selsum_t, e_full[:, sl], start=True, stop=True)
            nc.vector.reciprocal(rinv[:, sl], psd)
            nc.vector.tensor_mul(en[:, sl], e_full[:, sl], rinv[:, sl])

        def shifts(sb):
            """et[16j+h, s] = en[16j+h, s+j] for s-block sb — one DMA per j.

            Engine copies can't start at partition 16j (BIR rule: starts must
            be 0/32/64/96) and SBUF DMA APs need pitch-exact partition steps,
            so: plain 2-dim SBUF->SBUF DMAs, one per j-group.
            """
            c0 = 512 * sb
            for j in range(K):
                ln = 512 if sb < 3 else 512 - j
                nc.scalar.dma_start(
                    out=AP(tensor=et[:, :].tensor, offset=16 * j * S + c0,
                           ap=[[S, 16], [1, ln]]),
                    in_=AP(tensor=en[:, :].tensor, offset=16 * j * S + c0 + j,
                           ap=[[S, 16], [1, ln]]),
                )

        def t_chunks(lo, hi):
            for i in range(lo, hi):
                pt = p_tp.tile([128, D], BF16, tag="ptp")
                nc.tensor.transpose(
                    pt[:, :KH], et[:, 128 * i : 128 * (i + 1)],
                    identb_t[:KH, :KH],
                )
                nc.vector.tensor_copy(t_all[:, i, :], pt[:, :KH])

        dma1_done = set()

        def dma1(b):
            """T chunk b -> band image (skewed write; DRAM strides are free)."""
            if b in dma1_done or b >= NB:
                return
            dma1_done.add(b)
            nc.sync.dma_start(
                out=AP(tensor=img1[b % 4].tensor, offset=0,
                       ap=[[IMG_SKEW, 128], [1, KH]]),
                in_=t_all[:, b, :],
            )

        def block(b, ob, obhalf):
            """Banded conv for output rows 128b .. 128b+nl -> staging tile."""
            dma1(b + 2)
            if b + 1 < NB:
                # a2 tail: the left-guard cells of img1(b+1)
                nc.gpsimd.dma_start(
                    out=a2t[b % 2][:, A2LO : A2LO + KH],
                    in_=AP(tensor=img1[(b + 1) % 4].tensor, offset=0,
                           ap=[[IMG_PITCH, 6], [1, KH]]),
                )
            a1 = a1_pool.tile([128, ACOLS], BF16, tag="a1")
            nc.sync.dma_start(
                out=a1,
                in_=AP(tensor=img1[b % 4].tensor, offset=SLOT0,
                       ap=[[IMG_PITCH, 128], [1, ACOLS]]),
            )
            po = p_out.tile([128, D], F32, tag="pout")
            for h in range(H):
                stat1 = a1[:, h : h + 16 * 128 : 16]
                nc.tensor.matmul(
                    po[:, 64 * h : 64 * (h + 1)], stat1,
                    xb[b][:, h :: H],
                    start=True, stop=(b == NB - 1),
                )
                if b + 1 < NB:
                    stat2 = a2t[b % 2][:, h : h + 16 * 128 : 16]
                    nc.tensor.matmul(
                        po[:, 64 * h : 64 * (h + 1)], stat2,
                        xb[b + 1][:6, h :: H],
                        start=False, stop=True,
                    )
            # de-interleave h-major psum into natural channel order
            eng_copy = nc.scalar.copy if b % 2 == 0 else nc.vector.tensor_copy
            eng_copy(
                ob[:, 1024 * obhalf : 1024 * (obhalf + 1)].rearrange(
                    "p (dp h) -> p h dp", h=H
                ),
                po.rearrange("p (h dp) -> p h dp", h=H),
            )

        def run_pair(q):
            """Blocks 2q, 2q+1 -> one staging tile -> one (or two) stores."""
            ob = outs_pool.tile([128, 2 * D], F32, tag="outs")
            block(2 * q, ob, 0)
            block(2 * q + 1, ob, 1)
            r0 = 256 * q
            if q < 7:
                nc.scalar.dma_start(
                    out=AP(tensor=out_d.tensor, offset=r0 * D,
                           ap=[[D, 128], [128 * D, 2], [1, D]]),
                    in_=AP(tensor=ob[:, :].tensor, offset=0,
                           ap=[[2 * D, 128], [D, 2], [1, D]]),
                )
            else:
                nc.scalar.dma_start(
                    out=out_d[r0 : r0 + 128, :], in_=ob[:, :D]
                )
                nc.scalar.dma_start(
                    out=out_d[r0 + 128 : L, :], in_=ob[: L - r0 - 128, D:]
                )

        # ---- pipelined emission ----
        front(0)
        front(1)
        shifts(0)  # needs en cols [0, 512+6) -> after front(1)
        t_chunks(0, 4)
        dma1(0)
        dma1(1)
        front(2)
        shifts(1)
        t_chunks(4, 8)
        dma1(2)
        dma1(3)
        run_pair(0)  # blocks 0,1 (emit dma1 up to 4,5 via lookahead)
        run_pair(1)  # blocks 2,3
        front(3)
        shifts(2)
        t_chunks(8, 12)
        run_pair(2)
        run_pair(3)
        shifts(3)
        t_chunks(12, 16)
        for q in range(4, 8):
            run_pair(q)

    nc.compile()
    return nc


_CACHE = {}


def _get_program():
    if "nc" not in _CACHE:
        _CACHE["nc"] = build_program()
    return _CACHE["nc"]


def kernel(x, W, b):
    x = np.asarray(x, dtype=np.float32)
    assert x.shape == (B, S, D), x.shape

    nc = _get_program()
    consts = _host_constants(W, b)
    in_maps = []
    for core in range(B):
        in_maps.append(
            {
                "x": np.ascontiguousarray(x[core]),
                "consts": consts,
            }
        )
    res = bass_utils.run_bass_kernel_spmd(nc, in_maps, core_ids=list(range(B)))
    out = np.stack([res.results[core]["out"] for core in range(B)], axis=0)
    return out


# revision 26
# speedup vs baseline: 1.0695x; 1.0695x over previous
"""Dynamic lightweight convolution TRN2 kernel — banded-matmul design.

out[b,l,d] = (1/K) * sum_k softmax_k(x[b,l+K-1,:] @ W + bias)[k, d%H] * x[b,l+k,d]

B=8, S=2048, D=1024, K=7, H=16, L=S-K+1=2042.
Sharding: data-parallel over batch, one batch element per NeuronCore (8 cores).

Per-core plan — the conv itself runs on the *tensor engine* as banded-matrix
matmuls instead of elementwise DVE/GPSIMD work (which bottlenecked the old
design at ~104us busy per engine):

  1. x is loaded by GPSIMD (SWDGE) casting DMAs: f32 HBM -> bf16 SBUF chunks
     xb[i] [128, 1024] in natural [s, d] layout (halves input DMA bytes and
     removes the cast pass entirely).
  2. Logits path (from xb): PE-transpose xb -> xT per s-block, logits =
     W^T @ xT (PE, fp32 psum), e = exp(logits + bias) (ACT), denominators via
     a [112,112] selector matmul (PE), rinv = 1/. (DVE), en = e * rinv (DVE).
     W/bias columns are host-permuted k -> 6-k, so row 16j+h of en is the
     normalized weight of tap k = 6-j.
  3. Per-j-group shifted SBUF->SBUF DMAs build et[16j+h, s] = en[16j+h, s+j]
     (engine copies can't start at partition 16j, DMA can), then
     PE-transposes give T[s, r] (r = 16j+h), stored in T_all [128, chunk, 112].
  4. Band construction via a DRAM bounce (SBUF scatter DMAs cannot skew more
     than 128 bytes across partitions — hw descriptor drift limit — but DRAM
     strides are free): T_all[:, b] is written to a zero-filled DRAM image at
     skewed offsets IMG_SKEW*p + r and read back with row pitch IMG_PITCH,
     which lands T[p, r] at band position (p, 16p + r).  Non-band cells stay
     zero across blocks since each block overwrites exactly the same cells.
  5. Conv for 128-row output block b: for each h, a banded matmul
       out[l, d'] = sum_s A1_h[s, l] * xb[b][s, 16d'+h]
     with stationary A1_h = a1[:, h : h+2048 : 16] (h-interleaved band view).
     The 6-row contraction tail (s in the next chunk) uses a2: its band cells
     are exactly the *left guard* cells of img1(b+1), so a tiny [6, 112] load
     from img1(b+1) (rest of a2 is memset zero once) + a second matmul
     accumulating into the same psum.  PE cost is out-cols * 1 cyc/row only;
     LdWeights are free.
  6. psum [128, 1024] (h-major) -> SBUF staging with a de-interleaving copy
     (dst AP reorders 64h+d' -> 16d'+h); two blocks share one staging tile
     and one paired store DMA (3-dim DRAM dst AP).
"""

import numpy as np
import ml_dtypes
from contextlib import ExitStack

import concourse.bacc as bacc
import concourse.tile as tile
from concourse import mybir
from concourse import bass_utils
from concourse.ap import AP

K = 7
H = 16
B, S, D = 8, 2048, 1024
L = S - K + 1  # 2042
C = D // 128  # 8 d-chunks
NCH = S // 128  # 16 s-chunks
NB = 16  # output blocks of 128 rows (last has 122 valid)
KH = K * H  # 112

SLOT0 = 96  # img col of (l_rel=0, h=0): band tiles are loaded from this col
ACOLS = 2064  # band-tile cols needed by the stationary views
A2LO = 1952  # a2 col of (l_rel=122, h=0); cols below are zero
A2COLS = 2240  # a2 tile width: skew writes spill into cols >= 2064 (unread)
IMG_PITCH = 2256  # image read pitch (elements)
IMG_SKEW = IMG_PITCH + 16  # image write pitch: +16 elems (one slot) per row
IMG_ELEMS = IMG_PITCH * 128

F32 = mybir.dt.float32
BF16 = mybir.dt.bfloat16

# byte offsets (per partition) inside the packed constants blob
_OFF_BIAS = 0      # [112, 1] f32
_OFF_IDENTB = 4    # [128, 128] bf16
_OFF_SELSUM = 260  # [112, 112] bf16
_OFF_WT = 484      # [128, 8, 112] bf16
_CONST_BYTES = 2276  # 569 f32 columns


def _host_constants(W, b):
    """Pack bias/identb/selsum/W into one [128, 569] f32 blob."""
    buf = np.zeros((128, _CONST_BYTES), np.uint8)

    def put(off, arr):
        by = np.ascontiguousarray(arr).view(np.uint8).reshape(arr.shape[0], -1)
        buf[: arr.shape[0], off : off + by.shape[1]] = by

    # Permute the k-axis (k -> 6-k) of W and bias so that logits/e/en rows
    # come out in j-order (row 16j+h is the weight for tap k=6-j), matching
    # the band-image run layout r = 16j+h.
    perm = np.array([16 * (K - 1 - j) + h for j in range(K) for h in range(H)])
    put(_OFF_BIAS, np.asarray(b, np.float32)[perm].reshape(KH, 1))
    put(_OFF_IDENTB, np.eye(128).astype(ml_dtypes.bfloat16))
    hh = np.arange(KH) % H
    selsum = ((hh[:, None] == hh[None, :]) * float(K)).astype(ml_dtypes.bfloat16)
    put(_OFF_SELSUM, selsum)
    # W [D, KH] -> permuted -> [128, C, KH] chunks (d = c*128 + p)
    wt = np.asarray(W, np.float32)[:, perm].astype(ml_dtypes.bfloat16)
    wt = wt.reshape(C, 128, KH).transpose(1, 0, 2).reshape(128, C * KH)
    put(_OFF_WT, np.ascontiguousarray(wt))
    return buf.view(np.float32)


def build_program():
    nc = bacc.Bacc(
        "TRN2", target_bir_lowering=False, debug=False, enable_asserts=True
    )

    x_d = nc.dram_tensor("x", [S, D], F32, kind="ExternalInput").ap()
    consts_d = nc.dram_tensor(
        "consts", [128, _CONST_BYTES // 4], F32, kind="ExternalInput"
    ).ap()
    out_d = nc.dram_tensor("out", [L, D], F32, kind="ExternalOutput").ap()
    img1 = [
        nc.dram_tensor(f"img1{i}", [IMG_ELEMS], BF16, kind="Internal").ap()
        for i in range(4)
    ]

    with tile.TileContext(nc) as tc, ExitStack() as ctx:
        singles = ctx.enter_context(tc.tile_pool(name="singles", bufs=1))
        xT_pool = ctx.enter_context(tc.tile_pool(name="xT", bufs=2))
        a1_pool = ctx.enter_context(tc.tile_pool(name="a1", bufs=3))
        outs_pool = ctx.enter_context(tc.tile_pool(name="outs", bufs=2))

        p_tp = ctx.enter_context(tc.tile_pool(name="ptp", bufs=2, space="PSUM"))
        p_log = ctx.enter_context(tc.tile_pool(name="plog", bufs=1, space="PSUM"))
        p_sd = ctx.enter_context(tc.tile_pool(name="psd", bufs=1, space="PSUM"))
        p_out = ctx.enter_context(tc.tile_pool(name="pout", bufs=2, space="PSUM"))

        # ---- constants: one packed DMA, tiles are views into the blob ----
        cblob = singles.tile([128, _CONST_BYTES // 4], F32)
        nc.sync.dma_start(out=cblob, in_=consts_d)
        cbytes = cblob.bitcast(mybir.dt.uint8)

        def cview(off, nbytes, dt, rows=128):
            return cbytes[:rows, off : off + nbytes].bitcast(dt)

        bias_t = cview(_OFF_BIAS, 4, F32, rows=KH)
        identb_t = cview(_OFF_IDENTB, 256, BF16)
        selsum_t = cview(_OFF_SELSUM, 224, BF16, rows=KH)
        wt = cview(_OFF_WT, 1792, BF16).rearrange("p (c n) -> p c n", c=C)

        # GPSIMD ucode warmup
        warm = singles.tile([1, 8], BF16)
        nc.gpsimd.tensor_mul(warm, identb_t[:1, :8], identb_t[:1, :8])

        # ---- persistent tensors ----
        xb = [
            singles.tile([128, D], BF16, name=f"xb{i}") for i in range(NCH)
        ]
        e_full = singles.tile([KH, S], BF16)
        rinv = singles.tile([KH, S], F32)
        en = singles.tile([KH, S], BF16)
        et = singles.tile([KH, S], BF16)  # et[16j+h, s] = en[16j+h, s+j]
        t_all = singles.tile([128, NCH, KH], BF16)  # T[s, r], chunked
        zt = singles.tile([128, IMG_PITCH], BF16)  # zeros for image fill
        a2t = [
            singles.tile([6, A2COLS], BF16, name=f"a2t{i}") for i in range(2)
        ]

        # ---- prologue ----
        nc.vector.memset(zt, 0.0)
        # et tail cols: only read for invalid outputs l >= L; keep finite
        nc.vector.memset(et[:, S - 6 :], 0.0)
        # a2 tiles: cols < A2LO are always zero (out-of-band)
        nc.vector.memset(a2t[0], 0.0)
        nc.vector.memset(a2t[1], 0.0)
        for i in range(4):
            nc.sync.dma_start(
                out=AP(tensor=img1[i].tensor, offset=0,
                       ap=[[IMG_PITCH, 128], [1, IMG_PITCH]]),
                in_=zt[:, :],
            )
        # casting input DMAs (f32 HBM -> bf16 SBUF) via GPSIMD SWDGE
        for i in range(NCH):
            nc.gpsimd.dma_start(out=xb[i], in_=x_d[128 * i : 128 * (i + 1), :])

        # ---- stage helpers ----
        def front(sb):
            """Transpose chunks 4sb..4sb+3, logits, exp, denom, rinv, en."""
            sl = slice(512 * sb, 512 * (sb + 1))
            xTt = xT_pool.tile([128, C, 512], BF16, tag="xT")
            for q in range(4):
                i = 4 * sb + q
                ptp = p_tp.tile([128, D], BF16, tag="ptp")
                for c in range(C):
                    nc.tensor.transpose(
                        ptp[:, 128 * c : 128 * (c + 1)],
                        xb[i][:, 128 * c : 128 * (c + 1)],
                        identb_t,
                    )
                eng = nc.vector if q % 2 == 0 else nc.scalar
                cp = (eng.tensor_copy if q % 2 == 0 else eng.copy)
                cp(
                    xTt[:, :, 128 * q : 128 * (q + 1)],
                    ptp.rearrange("p (c s) -> p c s", c=C),
                )
            plog = p_log.tile([KH, 512], F32, tag="plog")
            for c in range(C):
                nc.tensor.matmul(
                    plog, wt[:, c, :], xTt[:, c, :],
                    start=(c == 0), stop=(c == C - 1),
                )
            nc.scalar.activation(
                e_full[:, sl], plog,
                mybir.ActivationFunctionType.Exp, bias=bias_t, scale=1.0,
            )
            psd = p_sd.tile([KH, 512], F32, tag="psd")
            nc.tensor.matmul(psd, selsum_t, e_full[:, sl], start=True, stop=True)
            nc.vector.reciprocal(rinv[:, sl], psd)
            nc.vector.tensor_mul(en[:, sl], e_full[:, sl], rinv[:, sl])

        def shifts(sb):
            """et[16j+h, s] = en[16j+h, s+j] for s-block sb — one DMA per j.

            Engine copies can't start at partition 16j (BIR rule: starts must
            be 0/32/64/96) and SBUF DMA APs need pitch-exact partition steps,
            so: plain 2-dim SBUF->SBUF DMAs, one per j-group.
            """
            c0 = 512 * sb
            for j in range(K):
                ln = 512 if sb < 3 else 512 - j
                nc.sync.dma_start(
                    out=AP(tensor=et[:, :].tensor, offset=16 * j * S + c0,
                           ap=[[S, 16], [1, ln]]),
                    in_=AP(tensor=en[:, :].tensor, offset=16 * j * S + c0 + j,
                           ap=[[S, 16], [1, ln]]),
                )

        def t_chunks(lo, hi):
            for i in range(lo, hi):
                pt = p_tp.tile([128, D], BF16, tag="ptp")
                nc.tensor.transpose(
                    pt[:, :KH], et[:, 128 * i : 128 * (i + 1)],
                    identb_t[:KH, :KH],
                )
                nc.vector.tensor_copy(t_all[:, i, :], pt[:, :KH])

        dma1_done = set()

        def dma1(b):
            """T chunk b -> band image (skewed write; DRAM strides are free)."""
            if b in dma1_done or b >= NB:
                return
            dma1_done.add(b)
            nc.sync.dma_start(
                out=AP(tensor=img1[b % 4].tensor, offset=0,
                       ap=[[IMG_SKEW, 128], [1, KH]]),
                in_=t_all[:, b, :],
            )

        dma2s_done = set()

        def dma2s(b):
            """a2 tail for block b: the left-guard cells of img1(b+1)."""
            if b in dma2s_done or b + 1 >= NB:
                return
            dma2s_done.add(b)
            nc.gpsimd.dma_start(
                out=a2t[b % 2][:, A2LO : A2LO + KH],
                in_=AP(tensor=img1[(b + 1) % 4].tensor, offset=0,
                       ap=[[IMG_PITCH, 6], [1, KH]]),
            )

        def block(b, ob, obhalf):
            """Banded conv for output rows 128b .. 128b+nl -> staging tile."""
            dma1(b + 2)
            dma2s(b)      # usually emitted one block earlier already
            dma2s(b + 1)  # needs img1(b+2), just emitted
            a1 = a1_pool.tile([128, ACOLS], BF16, tag="a1")
            nc.sync.dma_start(
                out=a1,
                in_=AP(tensor=img1[b % 4].tensor, offset=SLOT0,
                       ap=[[IMG_PITCH, 128], [1, ACOLS]]),
            )
            po = p_out.tile([128, D], F32, tag="pout")
            for h in range(H):
                stat1 = a1[:, h : h + 16 * 128 : 16]
                nc.tensor.matmul(
                    po[:, 64 * h : 64 * (h + 1)], stat1,
                    xb[b][:, h :: H],
                    start=True, stop=(b == NB - 1),
                )
                if b + 1 < NB:
                    stat2 = a2t[b % 2][:, h : h + 16 * 128 : 16]
                    nc.tensor.matmul(
                        po[:, 64 * h : 64 * (h + 1)], stat2,
                        xb[b + 1][:6, h :: H],
                        start=False, stop=True,
                    )
            # de-interleave h-major psum into natural channel order
            eng_copy = nc.scalar.copy if b % 2 == 0 else nc.vector.tensor_copy
            eng_copy(
                ob[:, 1024 * obhalf : 1024 * (obhalf + 1)].rearrange(
                    "p (dp h) -> p h dp", h=H
                ),
                po.rearrange("p (h dp) -> p h dp", h=H),
            )

        def run_pair(q):
            """Blocks 2q, 2q+1 -> one staging tile -> one (or two) stores."""
            ob = outs_pool.tile([128, 2 * D], F32, tag="outs")
            block(2 * q, ob, 0)
            block(2 * q + 1, ob, 1)
            r0 = 256 * q
            if q < 7:
                nc.scalar.dma_start(
                    out=AP(tensor=out_d.tensor, offset=r0 * D,
                           ap=[[D, 128], [128 * D, 2], [1, D]]),
                    in_=AP(tensor=ob[:, :].tensor, offset=0,
                           ap=[[2 * D, 128], [D, 2], [1, D]]),
                )
            else:
                nc.scalar.dma_start(
                    out=out_d[r0 : r0 + 128, :], in_=ob[:, :D]
                )
                nc.scalar.dma_start(
                    out=out_d[r0 + 128 : L, :], in_=ob[: L - r0 - 128, D:]
                )

        # ---- pipelined emission ----
        front(0)
        front(1)
        shifts(0)  # needs en cols [0, 512+6) -> after front(1)
        t_chunks(0, 4)
        dma1(0)
        dma1(1)
        front(2)
        shifts(1)
        t_chunks(4, 8)
        dma1(2)
        dma1(3)
        run_pair(0)  # blocks 0,1 (emit dma1 up to 4,5 via lookahead)
        run_pair(1)  # blocks 2,3
        front(3)
        shifts(2)
        t_chunks(8, 12)
        run_pair(2)
        run_pair(3)
        shifts(3)
        t_chunks(12, 16)
        for q in range(4, 8):
            run_pair(q)

    nc.compile()
    return nc


_CACHE = {}


def _get_program():
    if "nc" not in _CACHE:
        _CACHE["nc"] = build_program()
    return _CACHE["nc"]


def kernel(x, W, b):
    x = np.asarray(x, dtype=np.float32)
    assert x.shape == (B, S, D), x.shape

    nc = _get_program()
    consts = _host_constants(W, b)
    in_maps = []
    for core in range(B):
        in_maps.append(
            {
                "x": np.ascontiguousarray(x[core]),
                "consts": consts,
            }
        )
    res = bass_utils.run_bass_kernel_spmd(nc, in_maps, core_ids=list(range(B)))
    out = np.stack([res.results[core]["out"] for core in range(B)], axis=0)
    return out


# revision 32
# speedup vs baseline: 1.1651x; 1.0894x over previous
"""Dynamic lightweight convolution TRN2 kernel — banded-matmul design.

out[b,l,d] = (1/K) * sum_k softmax_k(x[b,l+K-1,:] @ W + bias)[k, d%H] * x[b,l+k,d]

B=8, S=2048, D=1024, K=7, H=16, L=S-K+1=2042.
Sharding: data-parallel over batch, one batch element per NeuronCore (8 cores).

Per-core plan — the conv itself runs on the *tensor engine* as banded-matrix
matmuls instead of elementwise DVE/GPSIMD work (which bottlenecked the old
design at ~104us busy per engine):

  1. x is loaded by GPSIMD (SWDGE) casting DMAs: f32 HBM -> bf16 SBUF chunks
     xb[i] [128, 1024] in natural [s, d] layout (halves input DMA bytes and
     removes the cast pass entirely).
  2. Logits path (from xb): PE-transpose xb -> xT per s-block, logits =
     W^T @ xT (PE, fp32 psum), e = exp(logits + bias) (ACT), denominators via
     a [112,112] selector matmul (PE), rinv = 1/. (DVE), en = e * rinv (DVE).
     W/bias columns are host-permuted k -> 6-k, so row 16j+h of en is the
     normalized weight of tap k = 6-j.
  3. Per-j-group shifted SBUF->SBUF DMAs build et[16j+h, s] = en[16j+h, s+j]
     (engine copies can't start at partition 16j, DMA can), then
     PE-transposes give T[s, r] (r = 16j+h), stored in T_all [128, chunk, 112].
  4. Band construction via a DRAM bounce (SBUF scatter DMAs cannot skew more
     than 128 bytes across partitions — hw descriptor drift limit — but DRAM
     strides are free): T_all[:, b] is written to a zero-filled DRAM image at
     skewed offsets IMG_SKEW*p + r and read back with row pitch IMG_PITCH,
     which lands T[p, r] at band position (p, 16p + r).  Non-band cells stay
     zero across blocks since each block overwrites exactly the same cells.
  5. Conv for 128-row output block b: for each h, a banded matmul
       out[l, d'] = sum_s A1_h[s, l] * xb[b][s, 16d'+h]
     with stationary A1_h = a1[:, h : h+2048 : 16] (h-interleaved band view).
     The 6-row contraction tail (s in the next chunk) uses a2: its band cells
     are exactly the *left guard* cells of img1(b+1), so a tiny [6, 112] load
     from img1(b+1) (rest of a2 is memset zero once) + a second matmul
     accumulating into the same psum.  PE cost is out-cols * 1 cyc/row only;
     LdWeights are free.
  6. psum [128, 1024] (h-major) -> SBUF staging with a de-interleaving copy
     (dst AP reorders 64h+d' -> 16d'+h); two blocks share one staging tile
     and one paired store DMA (3-dim DRAM dst AP).
"""

import numpy as np
import ml_dtypes
from contextlib import ExitStack

import concourse.bacc as bacc
import concourse.tile as tile
from concourse import mybir
from concourse import bass_utils
from concourse.ap import AP

K = 7
H = 16
B, S, D = 8, 2048, 1024
L = S - K + 1  # 2042
C = D // 128  # 8 d-chunks
NCH = S // 128  # 16 s-chunks
NB = 16  # output blocks of 128 rows (last has 122 valid)
KH = K * H  # 112

SLOT0 = 96  # img col of (l_rel=0, h=0): band tiles are loaded from this col
ACOLS = 2064  # band-tile cols needed by the stationary views
A2LO = 1952  # a2 col of (l_rel=122, h=0); cols below are zero
A2COLS = 2240  # a2 tile width: skew writes spill into cols >= 2064 (unread)
IMG_PITCH = 2256  # image read pitch (elements)
IMG_SKEW = IMG_PITCH + 16  # image write pitch: +16 elems (one slot) per row
IMG_ELEMS = IMG_PITCH * 128

F32 = mybir.dt.float32
BF16 = mybir.dt.bfloat16

# byte offsets (per partition) inside the packed constants blob
_OFF_BIAS = 0      # [112, 1] f32
_OFF_IDENTB = 4    # [128, 128] bf16
_OFF_SELSUM = 260  # [112, 112] bf16
_OFF_WT = 484      # [128, 8, 112] bf16
_CONST_BYTES = 2276  # 569 f32 columns


def _host_constants(W, b):
    """Pack bias/identb/selsum/W into one [128, 569] f32 blob."""
    buf = np.zeros((128, _CONST_BYTES), np.uint8)

    def put(off, arr):
        by = np.ascontiguousarray(arr).view(np.uint8).reshape(arr.shape[0], -1)
        buf[: arr.shape[0], off : off + by.shape[1]] = by

    # Permute the k-axis (k -> 6-k) of W and bias so that logits/e/en rows
    # come out in j-order (row 16j+h is the weight for tap k=6-j), matching
    # the band-image run layout r = 16j+h.
    perm = np.array([16 * (K - 1 - j) + h for j in range(K) for h in range(H)])
    put(_OFF_BIAS, np.asarray(b, np.float32)[perm].reshape(KH, 1))
    put(_OFF_IDENTB, np.eye(128).astype(ml_dtypes.bfloat16))
    hh = np.arange(KH) % H
    selsum = ((hh[:, None] == hh[None, :]) * float(K)).astype(ml_dtypes.bfloat16)
    put(_OFF_SELSUM, selsum)
    # W [D, KH] -> permuted -> [128, C, KH] chunks (d = c*128 + p)
    wt = np.asarray(W, np.float32)[:, perm].astype(ml_dtypes.bfloat16)
    wt = wt.reshape(C, 128, KH).transpose(1, 0, 2).reshape(128, C * KH)
    put(_OFF_WT, np.ascontiguousarray(wt))
    return buf.view(np.float32)


def build_program():
    nc = bacc.Bacc(
        "TRN2", target_bir_lowering=False, debug=False, enable_asserts=True
    )

    x_d = nc.dram_tensor("x", [S, D], F32, kind="ExternalInput").ap()
    consts_d = nc.dram_tensor(
        "consts", [128, _CONST_BYTES // 4], F32, kind="ExternalInput"
    ).ap()
    out_d = nc.dram_tensor("out", [L, D], F32, kind="ExternalOutput").ap()
    img1 = [
        nc.dram_tensor(f"img1{i}", [IMG_ELEMS], BF16, kind="Internal").ap()
        for i in range(3)
    ]

    with tile.TileContext(nc) as tc, ExitStack() as ctx:
        singles = ctx.enter_context(tc.tile_pool(name="singles", bufs=1))
        xT_pool = ctx.enter_context(tc.tile_pool(name="xT", bufs=2))
        a1_pool = ctx.enter_context(tc.tile_pool(name="a1", bufs=3))
        outs_pool = ctx.enter_context(tc.tile_pool(name="outs", bufs=2))

        p_tp = ctx.enter_context(tc.tile_pool(name="ptp", bufs=2, space="PSUM"))
        p_log = ctx.enter_context(tc.tile_pool(name="plog", bufs=1, space="PSUM"))
        p_sd = ctx.enter_context(tc.tile_pool(name="psd", bufs=1, space="PSUM"))
        p_out = ctx.enter_context(tc.tile_pool(name="pout", bufs=2, space="PSUM"))

        # ---- constants: one packed DMA, tiles are views into the blob ----
        cblob = singles.tile([128, _CONST_BYTES // 4], F32)
        nc.sync.dma_start(out=cblob, in_=consts_d)
        cbytes = cblob.bitcast(mybir.dt.uint8)

        def cview(off, nbytes, dt, rows=128):
            return cbytes[:rows, off : off + nbytes].bitcast(dt)

        bias_t = cview(_OFF_BIAS, 4, F32, rows=KH)
        identb_t = cview(_OFF_IDENTB, 256, BF16)
        selsum_t = cview(_OFF_SELSUM, 224, BF16, rows=KH)
        wt = cview(_OFF_WT, 1792, BF16).rearrange("p (c n) -> p c n", c=C)

        # GPSIMD ucode warmup
        warm = singles.tile([1, 8], BF16)
        nc.gpsimd.tensor_mul(warm, identb_t[:1, :8], identb_t[:1, :8])

        # ---- persistent tensors ----
        xb = [
            singles.tile([128, D], BF16, name=f"xb{i}") for i in range(NCH)
        ]
        e_full = singles.tile([KH, S], BF16)
        rinv = singles.tile([KH, S], F32)
        en = singles.tile([KH, S], BF16)
        et = singles.tile([KH, S], BF16)  # et[16j+h, s] = en[16j+h, s+j]
        t_all = singles.tile([128, NCH, KH], BF16)  # T[s, r], chunked
        zt = singles.tile([128, IMG_PITCH], BF16)  # zeros for image fill
        a2t = [
            singles.tile([6, A2COLS], BF16, name=f"a2t{i}") for i in range(2)
        ]

        # ---- prologue ----
        nc.vector.memset(zt, 0.0)
        # et tail cols: only read for invalid outputs l >= L; keep finite
        nc.vector.memset(et[:, S - 6 :], 0.0)
        # a2 tiles: cols < A2LO are always zero (out-of-band)
        nc.vector.memset(a2t[0], 0.0)
        nc.vector.memset(a2t[1], 0.0)
        # casting input DMAs (f32 HBM -> bf16 SBUF) via GPSIMD SWDGE
        for i in range(NCH):
            nc.gpsimd.dma_start(out=xb[i], in_=x_d[128 * i : 128 * (i + 1), :])

        # ---- stage helpers ----
        def front(sb):
            """Transpose chunks 4sb..4sb+3, logits, exp, denom, rinv, en.

            Logits matmuls run per 128-col chunk region (q-major, each
            region's 8 c-matmuls consecutive) so they start as soon as the
            first chunk's xT copy lands instead of after all four.
            """
            sl = slice(512 * sb, 512 * (sb + 1))
            xTt = xT_pool.tile([128, C, 512], BF16, tag="xT")
            for q in range(4):
                i = 4 * sb + q
                ptp = p_tp.tile([128, D], BF16, tag="ptp")
                for c in range(C):
                    nc.tensor.transpose(
                        ptp[:, 128 * c : 128 * (c + 1)],
                        xb[i][:, 128 * c : 128 * (c + 1)],
                        identb_t,
                    )
                eng = nc.vector if q % 2 == 0 else nc.scalar
                cp = (eng.tensor_copy if q % 2 == 0 else eng.copy)
                cp(
                    xTt[:, :, 128 * q : 128 * (q + 1)],
                    ptp.rearrange("p (c s) -> p c s", c=C),
                )
            plog = p_log.tile([KH, 512], F32, tag="plog")
            for q in range(4):
                for c in range(C):
                    nc.tensor.matmul(
                        plog[:, 128 * q : 128 * (q + 1)],
                        wt[:, c, :],
                        xTt[:, c, 128 * q : 128 * (q + 1)],
                        start=(c == 0), stop=(c == C - 1),
                    )
            nc.scalar.activation(
                e_full[:, sl], plog,
                mybir.ActivationFunctionType.Exp, bias=bias_t, scale=1.0,
            )
            psd = p_sd.tile([KH, 512], F32, tag="psd")
            nc.tensor.matmul(psd, selsum_t, e_full[:, sl], start=True, stop=True)
            nc.vector.reciprocal(rinv[:, sl], psd)
            nc.vector.tensor_mul(en[:, sl], e_full[:, sl], rinv[:, sl])

        def shifts(sb):
            """et[16j+h, s] = en[16j+h, s+j] for s-block sb — one DMA per j.

            Engine copies can't start at partition 16j (BIR rule: starts must
            be 0/32/64/96) and SBUF DMA APs need pitch-exact partition steps,
            so: plain 2-dim SBUF->SBUF DMAs, one per j-group.
            """
            c0 = 512 * sb
            for j in range(K):
                ln = 512 if sb < 3 else 512 - j
                nc.sync.dma_start(
                    out=AP(tensor=et[:, :].tensor, offset=16 * j * S + c0,
                           ap=[[S, 16], [1, ln]]),
                    in_=AP(tensor=en[:, :].tensor, offset=16 * j * S + c0 + j,
                           ap=[[S, 16], [1, ln]]),
                )

        def t_chunks(lo, hi):
            for i in range(lo, hi):
                pt = p_tp.tile([128, D], BF16, tag="ptp")
                nc.tensor.transpose(
                    pt[:, :KH], et[:, 128 * i : 128 * (i + 1)],
                    identb_t[:KH, :KH],
                )
                nc.vector.tensor_copy(t_all[:, i, :], pt[:, :KH])

        dma1_done = set()

        def dma1(b):
            """T chunk b -> band image (skewed write; DRAM strides are free)."""
            if b in dma1_done or b >= NB:
                return
            dma1_done.add(b)
            nc.sync.dma_start(
                out=AP(tensor=img1[b % 3].tensor, offset=0,
                       ap=[[IMG_SKEW, 128], [1, KH]]),
                in_=t_all[:, b, :],
            )

        dma2s_done = set()

        def dma2s(b):
            """a2 tail for block b: the left-guard cells of img1(b+1)."""
            if b in dma2s_done or b + 1 >= NB:
                return
            dma2s_done.add(b)
            nc.gpsimd.dma_start(
                out=a2t[b % 2][:, A2LO : A2LO + KH],
                in_=AP(tensor=img1[(b + 1) % 3].tensor, offset=0,
                       ap=[[IMG_PITCH, 6], [1, KH]]),
            )

        def block(b, ob, obhalf):
            """Banded conv for output rows 128b .. 128b+nl -> staging tile."""
            dma1(b + 2)
            dma2s(b)      # usually emitted one block earlier already
            dma2s(b + 1)  # needs img1(b+2), just emitted
            a1 = a1_pool.tile([128, ACOLS], BF16, tag="a1")
            nc.sync.dma_start(
                out=a1,
                in_=AP(tensor=img1[b % 3].tensor, offset=SLOT0,
                       ap=[[IMG_PITCH, 128], [1, ACOLS]]),
            )
            po = p_out.tile([128, D], F32, tag="pout")
            for h in range(H):
                stat1 = a1[:, h : h + 16 * 128 : 16]
                nc.tensor.matmul(
                    po[:, 64 * h : 64 * (h + 1)], stat1,
                    xb[b][:, h :: H],
                    start=True, stop=(b == NB - 1),
                )
                if b + 1 < NB:
                    stat2 = a2t[b % 2][:, h : h + 16 * 128 : 16]
                    nc.tensor.matmul(
                        po[:, 64 * h : 64 * (h + 1)], stat2,
                        xb[b + 1][:6, h :: H],
                        start=False, stop=True,
                    )
            # de-interleave h-major psum into natural channel order
            eng_copy = nc.scalar.copy if b % 2 == 0 else nc.vector.tensor_copy
            eng_copy(
                ob[:, 1024 * obhalf : 1024 * (obhalf + 1)].rearrange(
                    "p (dp h) -> p h dp", h=H
                ),
                po.rearrange("p (h dp) -> p h dp", h=H),
            )

        def run_pair(q):
            """Blocks 2q, 2q+1 -> one staging tile -> one (or two) stores."""
            ob = outs_pool.tile([128, 2 * D], F32, tag="outs")
            block(2 * q, ob, 0)
            block(2 * q + 1, ob, 1)
            r0 = 256 * q
            if q < 7:
                nc.scalar.dma_start(
                    out=AP(tensor=out_d.tensor, offset=r0 * D,
                           ap=[[D, 128], [128 * D, 2], [1, D]]),
                    in_=AP(tensor=ob[:, :].tensor, offset=0,
                           ap=[[2 * D, 128], [D, 2], [1, D]]),
                )
            else:
                nc.scalar.dma_start(
                    out=out_d[r0 : r0 + 128, :], in_=ob[:, :D]
                )
                nc.scalar.dma_start(
                    out=out_d[r0 + 128 : L, :], in_=ob[: L - r0 - 128, D:]
                )

        # ---- pipelined emission ----
        front(0)
        # img zero-fills: needed before dma1(0..); emitted after front(0) so
        # they don't compete with the input DMAs that gate the front pipeline
        for i in range(3):
            nc.sync.dma_start(
                out=AP(tensor=img1[i].tensor, offset=0,
                       ap=[[IMG_PITCH, 128], [1, IMG_PITCH]]),
                in_=zt[:, :],
            )
        front(1)
        shifts(0)  # needs en cols [0, 512+6) -> after front(1)
        t_chunks(0, 4)
        dma1(0)
        dma1(1)
        front(2)
        shifts(1)
        t_chunks(4, 8)
        run_pair(0)  # blocks 0,1 (emit dma1 lookahead b+2)
        run_pair(1)  # blocks 2,3
        front(3)
        shifts(2)
        t_chunks(8, 12)
        run_pair(2)
        run_pair(3)
        shifts(3)
        t_chunks(12, 16)
        for q in range(4, 8):
            run_pair(q)

    nc.compile()
    return nc


_CACHE = {}


def _get_program():
    if "nc" not in _CACHE:
        _CACHE["nc"] = build_program()
    return _CACHE["nc"]


def kernel(x, W, b):
    x = np.asarray(x, dtype=np.float32)
    assert x.shape == (B, S, D), x.shape

    nc = _get_program()
    consts = _host_constants(W, b)
    in_maps = []
    for core in range(B):
        in_maps.append(
            {
                "x": np.ascontiguousarray(x[core]),
                "consts": consts,
            }
        )
    res = bass_utils.run_bass_kernel_spmd(nc, in_maps, core_ids=list(range(B)))
    out = np.stack([res.results[core]["out"] for core in range(B)], axis=0)
    return out


# revision 34
# speedup vs baseline: 1.1698x; 1.0040x over previous
"""Dynamic lightweight convolution TRN2 kernel — banded-matmul design.

out[b,l,d] = (1/K) * sum_k softmax_k(x[b,l+K-1,:] @ W + bias)[k, d%H] * x[b,l+k,d]

B=8, S=2048, D=1024, K=7, H=16, L=S-K+1=2042.
Sharding: data-parallel over batch, one batch element per NeuronCore (8 cores).

Per-core plan — the conv itself runs on the *tensor engine* as banded-matrix
matmuls instead of elementwise DVE/GPSIMD work (which bottlenecked the old
design at ~104us busy per engine):

  1. x is loaded by GPSIMD (SWDGE) casting DMAs: f32 HBM -> bf16 SBUF chunks
     xb[i] [128, 1024] in natural [s, d] layout (halves input DMA bytes and
     removes the cast pass entirely).
  2. Logits path (from xb): PE-transpose xb -> xT per s-block, logits =
     W^T @ xT (PE, fp32 psum), e = exp(logits + bias) (ACT), denominators via
     a [112,112] selector matmul (PE), rinv = 1/. (DVE), en = e * rinv (DVE).
     W/bias columns are host-permuted k -> 6-k, so row 16j+h of en is the
     normalized weight of tap k = 6-j.
  3. Per-j-group shifted SBUF->SBUF DMAs build et[16j+h, s] = en[16j+h, s+j]
     (engine copies can't start at partition 16j, DMA can), then
     PE-transposes give T[s, r] (r = 16j+h), stored in T_all [128, chunk, 112].
  4. Band construction via a DRAM bounce (SBUF scatter DMAs cannot skew more
     than 128 bytes across partitions — hw descriptor drift limit — but DRAM
     strides are free): T_all[:, b] is written to a zero-filled DRAM image at
     skewed offsets IMG_SKEW*p + r and read back with row pitch IMG_PITCH,
     which lands T[p, r] at band position (p, 16p + r).  Non-band cells stay
     zero across blocks since each block overwrites exactly the same cells.
  5. Conv for 128-row output block b: for each h, a banded matmul
       out[l, d'] = sum_s A1_h[s, l] * xb[b][s, 16d'+h]
     with stationary A1_h = a1[:, h : h+2048 : 16] (h-interleaved band view).
     The 6-row contraction tail (s in the next chunk) uses a2: its band cells
     are exactly the *left guard* cells of img1(b+1), so a tiny [6, 112] load
     from img1(b+1) (rest of a2 is memset zero once) + a second matmul
     accumulating into the same psum.  PE cost is out-cols * 1 cyc/row only;
     LdWeights are free.
  6. psum [128, 1024] (h-major) -> SBUF staging with a de-interleaving copy
     (dst AP reorders 64h+d' -> 16d'+h); two blocks share one staging tile
     and one paired store DMA (3-dim DRAM dst AP).
"""

import numpy as np
import ml_dtypes
from contextlib import ExitStack

import concourse.bacc as bacc
import concourse.tile as tile
from concourse import mybir
from concourse import bass_utils
from concourse.ap import AP

K = 7
H = 16
B, S, D = 8, 2048, 1024
L = S - K + 1  # 2042
C = D // 128  # 8 d-chunks
NCH = S // 128  # 16 s-chunks
NB = 16  # output blocks of 128 rows (last has 122 valid)
KH = K * H  # 112

SLOT0 = 96  # img col of (l_rel=0, h=0): band tiles are loaded from this col
ACOLS = 2064  # band-tile cols needed by the stationary views
A2LO = 1952  # a2 col of (l_rel=122, h=0); cols below are zero
A2COLS = 2240  # a2 tile width: skew writes spill into cols >= 2064 (unread)
IMG_PITCH = 2256  # image read pitch (elements)
IMG_SKEW = IMG_PITCH + 16  # image write pitch: +16 elems (one slot) per row
IMG_ELEMS = IMG_PITCH * 128

F32 = mybir.dt.float32
BF16 = mybir.dt.bfloat16

# byte offsets (per partition) inside the packed constants blob
_OFF_BIAS = 0      # [112, 1] f32
_OFF_IDENTB = 4    # [128, 128] bf16
_OFF_SELSUM = 260  # [112, 112] bf16
_OFF_WT = 484      # [128, 8, 112] bf16
_CONST_BYTES = 2276  # 569 f32 columns


def _host_constants(W, b):
    """Pack bias/identb/selsum/W into one [128, 569] f32 blob."""
    buf = np.zeros((128, _CONST_BYTES), np.uint8)

    def put(off, arr):
        by = np.ascontiguousarray(arr).view(np.uint8).reshape(arr.shape[0], -1)
        buf[: arr.shape[0], off : off + by.shape[1]] = by

    # Permute the k-axis (k -> 6-k) of W and bias so that logits/e/en rows
    # come out in j-order (row 16j+h is the weight for tap k=6-j), matching
    # the band-image run layout r = 16j+h.
    perm = np.array([16 * (K - 1 - j) + h for j in range(K) for h in range(H)])
    put(_OFF_BIAS, np.asarray(b, np.float32)[perm].reshape(KH, 1))
    put(_OFF_IDENTB, np.eye(128).astype(ml_dtypes.bfloat16))
    hh = np.arange(KH) % H
    selsum = ((hh[:, None] == hh[None, :]) * float(K)).astype(ml_dtypes.bfloat16)
    put(_OFF_SELSUM, selsum)
    # W [D, KH] -> permuted -> [128, C, KH] chunks (d = c*128 + p)
    wt = np.asarray(W, np.float32)[:, perm].astype(ml_dtypes.bfloat16)
    wt = wt.reshape(C, 128, KH).transpose(1, 0, 2).reshape(128, C * KH)
    put(_OFF_WT, np.ascontiguousarray(wt))
    return buf.view(np.float32)


def build_program():
    nc = bacc.Bacc(
        "TRN2", target_bir_lowering=False, debug=False, enable_asserts=True
    )

    x_d = nc.dram_tensor("x", [S, D], F32, kind="ExternalInput").ap()
    consts_d = nc.dram_tensor(
        "consts", [128, _CONST_BYTES // 4], F32, kind="ExternalInput"
    ).ap()
    out_d = nc.dram_tensor("out", [L, D], F32, kind="ExternalOutput").ap()
    img1 = [
        nc.dram_tensor(f"img1{i}", [IMG_ELEMS], BF16, kind="Internal").ap()
        for i in range(3)
    ]

    with tile.TileContext(nc) as tc, ExitStack() as ctx:
        singles = ctx.enter_context(tc.tile_pool(name="singles", bufs=1))
        xT_pool = ctx.enter_context(tc.tile_pool(name="xT", bufs=3))
        a1_pool = ctx.enter_context(tc.tile_pool(name="a1", bufs=4))
        outs_pool = ctx.enter_context(tc.tile_pool(name="outs", bufs=3))

        p_tp = ctx.enter_context(tc.tile_pool(name="ptp", bufs=3, space="PSUM"))
        p_log = ctx.enter_context(tc.tile_pool(name="plog", bufs=1, space="PSUM"))
        p_out = ctx.enter_context(tc.tile_pool(name="pout", bufs=2, space="PSUM"))

        # ---- constants: one packed DMA, tiles are views into the blob ----
        cblob = singles.tile([128, _CONST_BYTES // 4], F32)
        nc.sync.dma_start(out=cblob, in_=consts_d)
        cbytes = cblob.bitcast(mybir.dt.uint8)

        def cview(off, nbytes, dt, rows=128):
            return cbytes[:rows, off : off + nbytes].bitcast(dt)

        bias_t = cview(_OFF_BIAS, 4, F32, rows=KH)
        identb_t = cview(_OFF_IDENTB, 256, BF16)
        selsum_t = cview(_OFF_SELSUM, 224, BF16, rows=KH)
        wt = cview(_OFF_WT, 1792, BF16).rearrange("p (c n) -> p c n", c=C)

        # GPSIMD ucode warmup
        warm = singles.tile([1, 8], BF16)
        nc.gpsimd.tensor_mul(warm, identb_t[:1, :8], identb_t[:1, :8])

        # ---- persistent tensors ----
        xb = [
            singles.tile([128, D], BF16, name=f"xb{i}") for i in range(NCH)
        ]
        e_full = singles.tile([KH, S], BF16)
        rinv = singles.tile([KH, S], F32)
        en = singles.tile([KH, S], BF16)
        et = singles.tile([KH, S], BF16)  # et[16j+h, s] = en[16j+h, s+j]
        t_all = singles.tile([128, NCH, KH], BF16)  # T[s, r], chunked
        zt = singles.tile([128, IMG_PITCH], BF16)  # zeros for image fill
        a2t = [
            singles.tile([6, A2COLS], BF16, name=f"a2t{i}") for i in range(2)
        ]

        # ---- prologue ----
        nc.vector.memset(zt, 0.0)
        # et tail cols: only read for invalid outputs l >= L; keep finite
        nc.vector.memset(et[:, S - 6 :], 0.0)
        # a2 tiles: cols < A2LO are always zero (out-of-band)
        nc.vector.memset(a2t[0], 0.0)
        nc.vector.memset(a2t[1], 0.0)
        # casting input DMAs (f32 HBM -> bf16 SBUF) via GPSIMD SWDGE
        for i in range(NCH):
            nc.gpsimd.dma_start(out=xb[i], in_=x_d[128 * i : 128 * (i + 1), :])

        # ---- stage helpers ----
        def front(sb):
            """Transpose chunks 4sb..4sb+3, logits, exp, denom, rinv, en.

            Logits matmuls run per 128-col chunk region (q-major, each
            region's 8 c-matmuls consecutive) so they start as soon as the
            first chunk's xT copy lands instead of after all four.
            """
            sl = slice(512 * sb, 512 * (sb + 1))
            xTt = xT_pool.tile([128, C, 512], BF16, tag="xT")
            for q in range(4):
                i = 4 * sb + q
                ptp = p_tp.tile([128, D], BF16, tag="ptp")
                for c in range(C):
                    nc.tensor.transpose(
                        ptp[:, 128 * c : 128 * (c + 1)],
                        xb[i][:, 128 * c : 128 * (c + 1)],
                        identb_t,
                    )
                eng = nc.vector if q % 2 == 0 else nc.scalar
                cp = (eng.tensor_copy if q % 2 == 0 else eng.copy)
                cp(
                    xTt[:, :, 128 * q : 128 * (q + 1)],
                    ptp.rearrange("p (c s) -> p c s", c=C),
                )
            plog = p_log.tile([KH, 512], F32, tag="plog")
            for q in range(4):
                for c in range(C):
                    nc.tensor.matmul(
                        plog[:, 128 * q : 128 * (q + 1)],
                        wt[:, c, :],
                        xTt[:, c, 128 * q : 128 * (q + 1)],
                        start=(c == 0), stop=(c == C - 1),
                    )
            nc.scalar.activation(
                e_full[:, sl], plog,
                mybir.ActivationFunctionType.Exp, bias=bias_t, scale=1.0,
            )
            psd = p_log.tile([KH, 512], F32, tag="plog")
            nc.tensor.matmul(psd, selsum_t, e_full[:, sl], start=True, stop=True)
            nc.vector.reciprocal(rinv[:, sl], psd)
            nc.vector.tensor_mul(en[:, sl], e_full[:, sl], rinv[:, sl])

        def shifts(sb):
            """et[16j+h, s] = en[16j+h, s+j] for s-block sb — one DMA per j.

            Engine copies can't start at partition 16j (BIR rule: starts must
            be 0/32/64/96) and SBUF DMA APs need pitch-exact partition steps,
            so: plain 2-dim SBUF->SBUF DMAs, one per j-group.
            """
            c0 = 512 * sb
            for j in range(K):
                ln = 512 if sb < 3 else 512 - j
                nc.sync.dma_start(
                    out=AP(tensor=et[:, :].tensor, offset=16 * j * S + c0,
                           ap=[[S, 16], [1, ln]]),
                    in_=AP(tensor=en[:, :].tensor, offset=16 * j * S + c0 + j,
                           ap=[[S, 16], [1, ln]]),
                )

        def t_chunks(lo, hi):
            for i in range(lo, hi):
                pt = p_tp.tile([128, D], BF16, tag="ptp")
                nc.tensor.transpose(
                    pt[:, :KH], et[:, 128 * i : 128 * (i + 1)],
                    identb_t[:KH, :KH],
                )
                nc.vector.tensor_copy(t_all[:, i, :], pt[:, :KH])

        dma1_done = set()

        def dma1(b):
            """T chunk b -> band image (skewed write; DRAM strides are free)."""
            if b in dma1_done or b >= NB:
                return
            dma1_done.add(b)
            nc.sync.dma_start(
                out=AP(tensor=img1[b % 3].tensor, offset=0,
                       ap=[[IMG_SKEW, 128], [1, KH]]),
                in_=t_all[:, b, :],
            )

        dma2s_done = set()

        def dma2s(b):
            """a2 tail for block b: the left-guard cells of img1(b+1)."""
            if b in dma2s_done or b + 1 >= NB:
                return
            dma2s_done.add(b)
            nc.gpsimd.dma_start(
                out=a2t[b % 2][:, A2LO : A2LO + KH],
                in_=AP(tensor=img1[(b + 1) % 3].tensor, offset=0,
                       ap=[[IMG_PITCH, 6], [1, KH]]),
            )

        def block(b, ob, obhalf):
            """Banded conv for output rows 128b .. 128b+nl -> staging tile."""
            dma1(b + 2)
            dma2s(b)      # usually emitted one block earlier already
            dma2s(b + 1)  # needs img1(b+2), just emitted
            a1 = a1_pool.tile([128, ACOLS], BF16, tag="a1")
            nc.sync.dma_start(
                out=a1,
                in_=AP(tensor=img1[b % 3].tensor, offset=SLOT0,
                       ap=[[IMG_PITCH, 128], [1, ACOLS]]),
            )
            po = p_out.tile([128, D], F32, tag="pout")
            for h in range(H):
                stat1 = a1[:, h : h + 16 * 128 : 16]
                nc.tensor.matmul(
                    po[:, 64 * h : 64 * (h + 1)], stat1,
                    xb[b][:, h :: H],
                    start=True, stop=(b == NB - 1),
                )
                if b + 1 < NB:
                    stat2 = a2t[b % 2][:, h : h + 16 * 128 : 16]
                    nc.tensor.matmul(
                        po[:, 64 * h : 64 * (h + 1)], stat2,
                        xb[b + 1][:6, h :: H],
                        start=False, stop=True,
                    )
            # de-interleave h-major psum into natural channel order
            eng_copy = nc.scalar.copy if b % 2 == 0 else nc.vector.tensor_copy
            eng_copy(
                ob[:, 1024 * obhalf : 1024 * (obhalf + 1)].rearrange(
                    "p (dp h) -> p h dp", h=H
                ),
                po.rearrange("p (h dp) -> p h dp", h=H),
            )

        def run_pair(q):
            """Blocks 2q, 2q+1 -> one staging tile -> one (or two) stores."""
            ob = outs_pool.tile([128, 2 * D], F32, tag="outs")
            block(2 * q, ob, 0)
            block(2 * q + 1, ob, 1)
            r0 = 256 * q
            if q < 7:
                nc.scalar.dma_start(
                    out=AP(tensor=out_d.tensor, offset=r0 * D,
                           ap=[[D, 128], [128 * D, 2], [1, D]]),
                    in_=AP(tensor=ob[:, :].tensor, offset=0,
                           ap=[[2 * D, 128], [D, 2], [1, D]]),
                )
            else:
                nc.scalar.dma_start(
                    out=out_d[r0 : r0 + 128, :], in_=ob[:, :D]
                )
                nc.scalar.dma_start(
                    out=out_d[r0 + 128 : L, :], in_=ob[: L - r0 - 128, D:]
                )

        # ---- pipelined emission ----
        front(0)
        # img zero-fills: needed before dma1(0..); emitted after front(0) so
        # they don't compete with the input DMAs that gate the front pipeline
        for i in range(3):
            nc.sync.dma_start(
                out=AP(tensor=img1[i].tensor, offset=0,
                       ap=[[IMG_PITCH, 128], [1, IMG_PITCH]]),
                in_=zt[:, :],
            )
        front(1)
        shifts(0)  # needs en cols [0, 512+6) -> after front(1)
        t_chunks(0, 4)
        dma1(0)
        dma1(1)
        front(2)
        shifts(1)
        t_chunks(4, 8)
        run_pair(0)  # blocks 0,1
        run_pair(1)  # blocks 2,3
        front(3)
        shifts(2)
        t_chunks(8, 12)
        run_pair(2)
        run_pair(3)
        shifts(3)
        t_chunks(12, 16)
        for q in range(4, 8):
            run_pair(q)

    nc.compile()
    return nc


_CACHE = {}


def _get_program():
    if "nc" not in _CACHE:
        _CACHE["nc"] = build_program()
    return _CACHE["nc"]


def kernel(x, W, b):
    x = np.asarray(x, dtype=np.float32)
    assert x.shape == (B, S, D), x.shape

    nc = _get_program()
    consts = _host_constants(W, b)
    in_maps = []
    for core in range(B):
        in_maps.append(
            {
                "x": np.ascontiguousarray(x[core]),
                "consts": consts,
            }
        )
    res = bass_utils.run_bass_kernel_spmd(nc, in_maps, core_ids=list(range(B)))
    out = np.stack([res.results[core]["out"] for core in range(B)], axis=0)
    return out
